# revision 38
# baseline (speedup 1.0000x reference)
"""Distributed CLIP-style loss (l2i symmetric CE + g2i NT-Xent) on 8 TRN2 cores.

v2: fp8 DoubleRow matmuls + circulant-banded symmetric g2i.

Each core k owns 256 l2i rows and 512 z rows. Inputs are column-ROTATED
transposed matrices (rotation = the core's global row offset), so one SPMD
program serves all 8 cores and the similarity structure becomes circulant:
the "upper triangle" of the symmetric (4096,4096) g2i sim matrix is, for
row-tile t, the contiguous local column band [128t, 128t+2176). Each pair
{i,j} is computed exactly once (d=(j-i)%4096 in [1,2048]); row-sums get the
missing lower-triangle terms from column sums of the exp matrix, assembled
on the host (d=2048 positive pairs land twice and are subtracted there).

g2i runs on RAW fp8 z (sim = G * rowinv_i * colinv_j applied after the
matmul with one fused scalar_tensor_tensor per psum half), so the sim
matmuls do not wait for the normalization pipeline. Norms come from a
host-shipped fp8 z^2 tensor via a DoubleRow ones-matmul + fast reciprocal.
Column sums of exp use weight-stationary matmuls (E-block as lhsT, ones
column as rhs) accumulating into a single [128,20] psum column so they
come out in partition layout with no cross-bank drains.
"""

import numpy as np
import ml_dtypes

import concourse.bass as bass
import concourse.mybir as mybir
from concourse.tile import TileContext
from concourse import bass_utils


# --- compat patches for the walrus build in this container ---------------
# 1) EVENT_SEMAPHORE_RANGE_CLEAR (InstISA op 176) is rejected ("ISA wrong
#    length"); emit one EventSemaphore sem-wr-imm 0 per semaphore instead.
SEM_CLEAR_BATCH = 1  # walrus rejects >1 sem update per EventSemaphore


def _sem_clear_compat(self, sem):
    nums = list(sem) if isinstance(sem, range) else [
        sem.num if hasattr(sem, "num") else int(sem)
    ]
    last = None
    for i in range(0, len(nums), SEM_CLEAR_BATCH):
        last = self.add_instruction(
            mybir.InstEventSemaphore(
                name=self.bass.get_next_instruction_name(),
                ins=[], outs=[],
                sync_info=mybir.SyncInfo(
                    on_wait=[],
                    on_update=[mybir.SyncUpdate(
                        sync_type="semaphore", id=n,
                        update_mode="sem-wr-imm", update_value=0)
                        for n in nums[i:i + SEM_CLEAR_BATCH]],
                ),
            )
        )
    return last


bass.BassGpSimd.sem_clear = _sem_clear_compat


# 2) Every instruction in this walrus build has a single sync-wait slot
#    ("Too many sync wait commands" otherwise), while Tile freely attaches
#    several. Post-pass: hoist extra waits onto wait-only EventSemaphore
#    instructions inserted immediately before the instruction on the same
#    engine (sequencers execute in order, so the semantics are identical).
_mw_ctr = [0]


def _split_multi_waits(nc: bass.Bass) -> None:
    for f in nc.m.functions:
        for bb in f.blocks:
            out = []
            changed = False
            for inst in bb.instructions:
                si = inst.sync_info
                waits = list(si.on_wait) if si is not None and si.on_wait else []
                if len(waits) > 1:
                    for w in waits[:-1]:
                        _mw_ctr[0] += 1
                        es = mybir.InstEventSemaphore(
                            name=f"I-mwsplit-{_mw_ctr[0]}",
                            engine=inst.engine,
                            ins=[], outs=[],
                            sync_info=mybir.SyncInfo(on_wait=[w], on_update=[]),
                        )
                        out.append(es)
                    inst.sync_info = mybir.SyncInfo(
                        on_wait=[waits[-1]],
                        on_update=list(si.on_update or []),
                    )
                    changed = True
                out.append(inst)
            if changed:
                bb.instructions = out
# -------------------------------------------------------------------------

B = 2048
D = 1024
N = 2 * B                  # 4096 z rows
NCORES = 8
TEMP = 0.05
INV_TEMP = 1.0 / TEMP
BPC = B // NCORES          # 256 image/text rows per core
ZPC = N // NCORES          # 512 z rows per core
NCP = D // 256             # 4 DoubleRow chunk-pairs
BAND = 2048 + 128          # g2i band width per row-tile
ZCOLS = 128 * 3 + BAND     # 2560 local z columns each core touches
NT_L = BPC // 128          # 2 l2i row-tiles per core
NT_G = ZPC // 128          # 4 g2i row-tiles per core
NBLK = ZCOLS // 128        # 20 column blocks for colacc

F8 = mybir.dt.float8e4
BF16 = mybir.dt.bfloat16
F32 = mybir.dt.float32
AF = mybir.ActivationFunctionType
ALU = mybir.AluOpType
DR = mybir.MatmulPerfMode.DoubleRow

# stats_out column layout ([128, 40] f32 per core)
COL_LMAX = 0    # + side*4 + t*2 + h   (8): per-half row max of raw dots
COL_LSUM = 8    # + side*4 + t*2 + h   (8): per-half sum exp(ls*(x-max))
COL_LPOS = 16   # + t                  (2): raw positive dot (unscaled)
COL_GSUM = 18   # + t*3 + {h0,h1,tail} (12): per-part sum exp(sim/temp)
COL_GPOS = 30   # + t                  (4): raw positive cosine sim

# fallback switches for instructions this walrus build rejects ("ISA wrong
# length"): tensor_tensor_reduce and reciprocal_approx_fast both die there.
USE_TTR = False  # tensor_tensor_reduce (else tensor_mul + reduce_sum)
USE_RAF = False  # reciprocal_approx_fast (else rsqrt = exp(-0.5*ln))
USE_STT = True   # scalar_tensor_tensor (else tensor_scalar + tensor_mul)

_cache: dict = {}


def _diag_extract(nc, workp, src, eye, accum):
    """accum[p] = src[p, p] via eye multiply + row reduce."""
    scr = workp.tile([128, 128], F32, tag="scr", bufs=2, name="scr")
    if USE_TTR:
        nc.vector.tensor_tensor_reduce(
            scr, src, eye, 1.0, 0.0, ALU.mult, ALU.add, accum)
    else:
        nc.vector.tensor_mul(scr, src, eye)
        nc.vector.reduce_sum(accum, scr, axis=mybir.AxisListType.X)


def _scale_rowcol(nc, workp, out, ps, rowinv, colinv):
    """out = ps * rowinv (per-partition) * colinv (per-column)."""
    if USE_STT:
        nc.vector.scalar_tensor_tensor(
            out, ps, rowinv, colinv, ALU.mult, ALU.mult)
    else:
        nc.vector.tensor_scalar_mul(out, ps, rowinv)
        nc.vector.tensor_mul(out, out, colinv)


def _build_program(ls: float) -> bass.Bass:
    nc = bass.Bass(trn_type="TRN2")
    # host pre-interleaves to [128, n_chunks*N] (partition-major) so each
    # chunk-pair (or whole tensor) moves in one contiguous DMA descriptor
    img_d = nc.dram_tensor("img", [128, 8 * B], F8, kind="ExternalInput")
    txt_d = nc.dram_tensor("txt", [128, 8 * B], F8, kind="ExternalInput")
    z_d = nc.dram_tensor("z", [128, 8 * ZCOLS], F8, kind="ExternalInput")
    zsq_d = nc.dram_tensor("zsq", [128, 8 * ZCOLS], F8, kind="ExternalInput")
    eye_d = nc.dram_tensor("eye", [128, 128], F32, kind="ExternalInput")
    mask0_d = nc.dram_tensor("mask0", [128, 128], F32, kind="ExternalInput")
    maskt_d = nc.dram_tensor("maskt", [128, 128], F32, kind="ExternalInput")
    stats_d = nc.dram_tensor("stats", [128, 40], F32, kind="ExternalOutput")
    colq_d = nc.dram_tensor("colq", [128, 4 * 17], F32, kind="ExternalOutput")

    with TileContext(nc) as tc:
        with (
            tc.tile_pool(name="consts", bufs=1) as consts,
            tc.tile_pool(name="feat", bufs=NCP) as featp,
            tc.tile_pool(name="escr", bufs=3) as escrp,
            tc.tile_pool(name="work", bufs=4) as workp,
            tc.tile_pool(name="mm", bufs=1, space="PSUM") as mmp,
        ):
            eye = consts.tile([128, 128], F32, tag="eye")
            mask0 = consts.tile([128, 128], F32, tag="mask0")
            maskt = consts.tile([128, 128], F32, tag="maskt")

            ones8 = consts.tile([128, 2, 128], F8, tag="ones8")
            onesb = consts.tile([128, 1], BF16, tag="onesb")
            nc.vector.memset(ones8, 1.0)
            nc.vector.memset(onesb, 1.0)

            stats = consts.tile([128, 40], F32, tag="stats")
            nc.vector.memset(stats, 0.0)

            invn = consts.tile([128, ZCOLS], F32, tag="invn")
            E = consts.tile([128, NT_G, BAND], BF16, tag="E")
            colq_s = consts.tile([128, 4 * 17], F32, tag="colqs")

            # ---- input DMAs ----
            # img/txt: one descriptor per chunk-pair on the sync queue so
            # l2i matmuls can start as pairs land
            img_c = []
            txt_c = []
            for cp in range(NCP):
                it = featp.tile([128, 2, B], F8, tag="img", name="it")
                tt = featp.tile([128, 2, B], F8, tag="tt", name="tt")
                if cp == 0:
                    # split the first pair chunk-wise so PE starts sooner
                    for i in range(2):
                        nc.sync.dma_start(
                            it[:, i, :], img_d[:, i * B:(i + 1) * B])
                        nc.sync.dma_start(
                            tt[:, i, :], txt_d[:, i * B:(i + 1) * B])
                else:
                    nc.sync.dma_start(
                        it, img_d[:, 2 * cp * B:(2 * cp + 2) * B])
                    nc.sync.dma_start(
                        tt, txt_d[:, 2 * cp * B:(2 * cp + 2) * B])
                img_c.append(it)
                txt_c.append(tt)
            # small consts after the first pair (needed only ~mid-l2i)
            nc.sync.dma_start(eye, eye_d[:, :])
            nc.sync.dma_start(mask0, mask0_d[:, :])
            nc.sync.dma_start(maskt, maskt_d[:, :])
            # zsq then z, pair-granular on the same sync queue (a second
            # HWDGE queue or whole-tensor descriptors both starve the
            # small img/txt transfers behind 5MB of z traffic)
            zsq_t = featp.tile([128, 8, ZCOLS], F8, tag="zsq", bufs=1,
                               name="zsq_t")
            z_t = featp.tile([128, 8, ZCOLS], F8, tag="z", bufs=1, name="z_t")
            for cp in range(NCP):
                nc.sync.dma_start(
                    zsq_t[:, 2 * cp:2 * cp + 2, :],
                    zsq_d[:, 2 * cp * ZCOLS:(2 * cp + 2) * ZCOLS])
            for cp in range(NCP):
                nc.sync.dma_start(
                    z_t[:, 2 * cp:2 * cp + 2, :],
                    z_d[:, 2 * cp * ZCOLS:(2 * cp + 2) * ZCOLS])
            zsq_c = [zsq_t[:, 2 * cp:2 * cp + 2, :] for cp in range(NCP)]
            z_c = [z_t[:, 2 * cp:2 * cp + 2, :] for cp in range(NCP)]

            colq = mmp.tile([128, 512], F32, tag="colq", bufs=1, name="colq")

            # ================= Phase A: l2i (two sides) =================
            # Per (side, row-tile, half): [128,1024] psum; per-half max and
            # exp-sum are exported raw and combined into lse on the host.
            # Groups run in waves of 2 with the chunk-pair loop OUTER so
            # each arriving img/txt pair triggers a dense matmul burst
            # (group-major order walks the DMA waits with one group's
            # sparse matmuls and keeps the PE HAM-throttled at K=4).
            groups = [(side, t, h) for side in range(2)
                      for t in range(NT_L) for h in range(2)]
            for w0 in range(0, len(groups), 2):
                wave = groups[w0:w0 + 2]
                pss = []
                for _ in wave:
                    ps = mmp.tile([128, 1024], F32, tag="big", bufs=3,
                                  name="ps")
                    pss.append(ps)
                for cp in range(NCP):
                    for gi, (side, t, h) in enumerate(wave):
                        lhs_c = img_c if side == 0 else txt_c
                        rhs_c = txt_c if side == 0 else img_c
                        for q in range(2):
                            nc.tensor.matmul(
                                pss[gi][:, q * 512:(q + 1) * 512],
                                lhs_c[cp][:, :, t * 128:(t + 1) * 128],
                                rhs_c[cp][:, :, h * 1024 + q * 512:
                                          h * 1024 + (q + 1) * 512],
                                start=(cp == 0), stop=(cp == NCP - 1),
                                perf_mode=DR,
                            )
                for gi, (side, t, h) in enumerate(wave):
                    ps = pss[gi]
                    if side == 0 and h == 0:
                        # raw positive dot: diag of the [128,128] block
                        _diag_extract(
                            nc, workp, ps[:, t * 128:(t + 1) * 128], eye,
                            stats[:, COL_LPOS + t:COL_LPOS + t + 1])
                    col = side * 4 + t * 2 + h
                    mx = stats[:, COL_LMAX + col:COL_LMAX + col + 1]
                    nc.vector.reduce_max(mx, ps, axis=mybir.AxisListType.X)
                    negb = workp.tile([128, 1], F32, tag="negb", name="negb")
                    nc.scalar.mul(negb, mx, -ls)
                    escr = escrp.tile([128, 1024], BF16, tag="escr",
                                      name="escr")
                    nc.scalar.activation(
                        escr, ps, AF.Exp, bias=negb, scale=ls,
                        accum_out=stats[:, COL_LSUM + col:
                                        COL_LSUM + col + 1],
                    )

            # ================= Phase B: z norms =================
            # nsq[j] = sum_d z[d,j]^2 broadcast to all partitions via a
            # DoubleRow ones-matmul over host-shipped zsq; invn = rsqrt.
            nwid = [1024, 1024, 512]
            for i in range(3):
                nb = mmp.tile([128, 1024], F32, tag="big", bufs=3, name="nb")
                for q in range(nwid[i] // 512):
                    off = i * 1024 + q * 512
                    for cp in range(NCP):
                        nc.tensor.matmul(
                            nb[:, q * 512:(q + 1) * 512],
                            ones8,
                            zsq_c[cp][:, :, off:off + 512],
                            start=(cp == 0), stop=(cp == NCP - 1),
                            perf_mode=DR,
                        )
                sl = slice(i * 1024, i * 1024 + nwid[i])
                if USE_RAF:
                    nc.vector.reciprocal_approx_fast(
                        invn[:, sl], nb[:, :nwid[i]])
                    nc.scalar.activation(invn[:, sl], invn[:, sl], AF.Sqrt)
                else:
                    # rsqrt(x) = exp(-0.5*ln(x)) on the accurate ACT tables
                    nc.scalar.activation(invn[:, sl], nb[:, :nwid[i]], AF.Ln)
                    nc.scalar.activation(invn[:, sl], invn[:, sl], AF.Exp,
                                         scale=-0.5)

            # ================= Phase C: g2i (banded symmetric) =================
            for t in range(NT_G):
                # inv-norms of this tile's own rows, in partition layout
                rowinv = workp.tile([128, 1], F32, tag="rinv", bufs=2,
                                    name="rowinv")
                _diag_extract(nc, workp, invn[:, t * 128:(t + 1) * 128],
                              eye, rowinv)
                for h in range(2):
                    ps = mmp.tile([128, 1024], F32, tag="big", bufs=3,
                                  name="ps")
                    for q in range(2):
                        off = t * 128 + h * 1024 + q * 512
                        for cp in range(NCP):
                            nc.tensor.matmul(
                                ps[:, q * 512:(q + 1) * 512],
                                z_c[cp][:, :, t * 128:(t + 1) * 128],
                                z_c[cp][:, :, off:off + 512],
                                start=(cp == 0), stop=(cp == NCP - 1),
                                perf_mode=DR,
                            )
                    # sim = G * rowinv_i * colinv_j, fused STT writing to
                    # SBUF so the psum slot frees after DVE (not after exp)
                    simS = escrp.tile([128, 1024], F32, tag="simS",
                                      name="simS")
                    _scale_rowcol(
                        nc, workp, simS, ps, rowinv,
                        invn[:, t * 128 + h * 1024:t * 128 + (h + 1) * 1024])
                    if h == 0:
                        # self-block: keep strict upper (d in [1,127])
                        nc.vector.tensor_add(
                            simS[:, 0:128], simS[:, 0:128], mask0)
                    col = COL_GSUM + t * 3 + h
                    nc.scalar.activation(
                        E[:, t, h * 1024:(h + 1) * 1024], simS, AF.Exp,
                        scale=INV_TEMP,
                        accum_out=stats[:, col:col + 1],
                    )
                # tail block: cols [2048, 2176) of the band, keep d<=2048
                pt = mmp.tile([128, 512], F32, tag="tail", bufs=1, name="pt")
                off = t * 128 + 2048
                for cp in range(NCP):
                    nc.tensor.matmul(
                        pt[:, 0:128],
                        z_c[cp][:, :, t * 128:(t + 1) * 128],
                        z_c[cp][:, :, off:off + 128],
                        start=(cp == 0), stop=(cp == NCP - 1),
                        perf_mode=DR,
                    )
                tailS = workp.tile([128, 128], F32, tag="tailS", bufs=2,
                                   name="tailS")
                _scale_rowcol(nc, workp, tailS, pt[:, 0:128], rowinv,
                              invn[:, off:off + 128])
                nc.vector.tensor_add(tailS, tailS, maskt)
                # positive pair: diag (d = 2048) of the tail block
                _diag_extract(nc, workp, tailS, eye,
                              stats[:, COL_GPOS + t:COL_GPOS + t + 1])
                col = COL_GSUM + t * 3 + 2
                nc.scalar.activation(
                    E[:, t, 2048:2048 + 128], tailS, AF.Exp,
                    scale=INV_TEMP,
                    accum_out=stats[:, col:col + 1],
                )

                # column sums: weight-stationary matmuls, E block as lhsT,
                # ones column as rhs -> per-(t,block) column in partition
                # layout. Single-shot matmuls (interleaved long-lived psum
                # accumulation groups lose prior contributions on HW); the
                # host sums the 4 tile layers.
                for j in range(17):
                    nc.tensor.matmul(
                        colq[:, t * 17 + j:t * 17 + j + 1],
                        E[:, t, j * 128:(j + 1) * 128],
                        onesb,
                    )

            # split the drains so the final chain only waits on tile-3 bits
            nc.scalar.copy(colq_s[:, 0:3 * 17], colq[:, 0:3 * 17])
            nc.sync.dma_start(stats_d[:, 0:COL_GSUM], stats[:, 0:COL_GSUM])
            nc.sync.dma_start(colq_d[:, 0:3 * 17], colq_s[:, 0:3 * 17])
            nc.scalar.copy(colq_s[:, 3 * 17:4 * 17], colq[:, 3 * 17:4 * 17])
            nc.sync.dma_start(stats_d[:, COL_GSUM:], stats[:, COL_GSUM:])
            nc.sync.dma_start(colq_d[:, 3 * 17:4 * 17], colq_s[:, 3 * 17:4 * 17])

    _split_multi_waits(nc)
    return nc


def _get_program(ls: float) -> bass.Bass:
    key = float(ls)
    if key not in _cache:
        _cache[key] = _build_program(key)
    return _cache[key]


def kernel(image_features, gli_features, text_features, logit_scale):
    ls = float(np.asarray(logit_scale))
    nc = _get_program(ls)

    f8 = ml_dtypes.float8_e4m3
    imgT = np.ascontiguousarray(np.asarray(image_features, np.float32).T)
    txtT = np.ascontiguousarray(np.asarray(text_features, np.float32).T)
    z = np.concatenate(
        [np.asarray(gli_features, np.float32),
         np.asarray(image_features, np.float32)], axis=0)
    zT = np.ascontiguousarray(z.T)

    eye = np.eye(128, dtype=np.float32)
    r = np.arange(128)
    # mask0: keep strict upper (s > r); maskt: keep s <= r (incl. diag)
    mask0 = np.where(r[None, :] > r[:, None], 0.0, -1e30).astype(np.float32)
    maskt = np.where(r[None, :] <= r[:, None], 0.0, -1e30).astype(np.float32)

    def interleave(a, ncols):
        # [1024, ncols] -> [128, 8*ncols] partition-major chunk layout
        return np.ascontiguousarray(
            a.reshape(8, 128, ncols).transpose(1, 0, 2).reshape(128, -1))

    in_maps = []
    for k in range(NCORES):
        zr = np.roll(zT, -ZPC * k, axis=1)[:, :ZCOLS].astype(f8)
        zrf = zr.astype(np.float32)
        in_maps.append({
            "img": interleave(np.roll(imgT, -BPC * k, axis=1).astype(f8), B),
            "txt": interleave(np.roll(txtT, -BPC * k, axis=1).astype(f8), B),
            "z": interleave(zr, ZCOLS),
            "zsq": interleave((zrf * zrf).astype(f8), ZCOLS),
            "eye": eye,
            "mask0": mask0,
            "maskt": maskt,
        })

    res = bass_utils.run_bass_kernel_spmd(nc, in_maps, core_ids=list(range(NCORES)))
    globals()["LAST_RESULT"] = res
    stats = np.stack([r_["stats"] for r_ in res.results]).astype(np.float64)
    colq = np.stack([r_["colq"] for r_ in res.results]).astype(np.float64)

    # ---- l2i: combine per-half (max, sumexp) into lse on the host ----
    lse_sum = np.zeros(2)
    for side in range(2):
        for t in range(NT_L):
            cols = [side * 4 + t * 2 + h for h in range(2)]
            m = stats[:, :, [COL_LMAX + c for c in cols]]      # [8,128,2]
            s = stats[:, :, [COL_LSUM + c for c in cols]]
            M = m.max(axis=2)
            comb = (s * np.exp(ls * (m - M[:, :, None]))).sum(axis=2)
            lse_sum[side] += (ls * M + np.log(comb)).sum()
    pos_l2i = stats[:, :, COL_LPOS:COL_LPOS + NT_L].sum()
    l2i = 0.5 * ((lse_sum[0] - ls * pos_l2i) / B
                 + (lse_sum[1] - ls * pos_l2i) / B)

    # ---- g2i: assemble row sums from row partials + column sums ----
    # per-core row partials rowacc[k, local_row]
    rowacc = np.zeros((NCORES, ZPC))
    pos = np.zeros((NCORES, ZPC))
    for t in range(NT_G):
        sl = slice(t * 128, (t + 1) * 128)
        rowacc[:, sl] = stats[:, :, COL_GSUM + t * 3:COL_GSUM + t * 3 + 3] \
            .sum(axis=2)
        pos[:, sl] = stats[:, :, COL_GPOS + t]
    # column sums: colq[k, i, t*17+j] is local column 128*(t+j) + i
    colsum = np.zeros(N)
    for k in range(NCORES):
        local = np.zeros(ZCOLS)
        for t in range(NT_G):
            lc = colq[k][:, t * 17:(t + 1) * 17]   # [128, 17]
            local[128 * t:128 * t + 2176] += lc.T.reshape(-1)
        gidx = (ZPC * k + np.arange(ZCOLS)) % N
        np.add.at(colsum, gidx, local)
    rows = rowacc.reshape(-1)
    posf = pos.reshape(-1)
    total = rows + colsum - np.exp(INV_TEMP * posf)
    lse = np.log(total)
    g2i = (lse - INV_TEMP * posf).sum() / N

    total_loss = l2i + g2i
    return (np.float32(total_loss), np.float32(l2i), np.float32(g2i))


# revision 40
# speedup vs baseline: 1.0295x; 1.0295x over previous
"""Distributed CLIP-style loss (l2i symmetric CE + g2i NT-Xent) on 8 TRN2 cores.

v2: fp8 DoubleRow matmuls + circulant-banded symmetric g2i.

Each core k owns 256 l2i rows and 512 z rows. Inputs are column-ROTATED
transposed matrices (rotation = the core's global row offset), so one SPMD
program serves all 8 cores and the similarity structure becomes circulant:
the "upper triangle" of the symmetric (4096,4096) g2i sim matrix is, for
row-tile t, the contiguous local column band [128t, 128t+2176). Each pair
{i,j} is computed exactly once (d=(j-i)%4096 in [1,2048]); row-sums get the
missing lower-triangle terms from column sums of the exp matrix, assembled
on the host (d=2048 positive pairs land twice and are subtracted there).

g2i runs on RAW fp8 z (sim = G * rowinv_i * colinv_j applied after the
matmul with one fused scalar_tensor_tensor per psum half), so the sim
matmuls do not wait for the normalization pipeline. Norms come from a
host-shipped fp8 z^2 tensor via a DoubleRow ones-matmul + fast reciprocal.
Column sums of exp use weight-stationary matmuls (E-block as lhsT, ones
column as rhs) accumulating into a single [128,20] psum column so they
come out in partition layout with no cross-bank drains.
"""

import numpy as np
import ml_dtypes

import concourse.bass as bass
import concourse.mybir as mybir
from concourse.tile import TileContext
from concourse import bass_utils


# --- compat patches for the walrus build in this container ---------------
# 1) EVENT_SEMAPHORE_RANGE_CLEAR (InstISA op 176) is rejected ("ISA wrong
#    length"); emit one EventSemaphore sem-wr-imm 0 per semaphore instead.
SEM_CLEAR_BATCH = 1  # walrus rejects >1 sem update per EventSemaphore


def _sem_clear_compat(self, sem):
    nums = list(sem) if isinstance(sem, range) else [
        sem.num if hasattr(sem, "num") else int(sem)
    ]
    last = None
    for i in range(0, len(nums), SEM_CLEAR_BATCH):
        last = self.add_instruction(
            mybir.InstEventSemaphore(
                name=self.bass.get_next_instruction_name(),
                ins=[], outs=[],
                sync_info=mybir.SyncInfo(
                    on_wait=[],
                    on_update=[mybir.SyncUpdate(
                        sync_type="semaphore", id=n,
                        update_mode="sem-wr-imm", update_value=0)
                        for n in nums[i:i + SEM_CLEAR_BATCH]],
                ),
            )
        )
    return last


bass.BassGpSimd.sem_clear = _sem_clear_compat


# 2) Every instruction in this walrus build has a single sync-wait slot
#    ("Too many sync wait commands" otherwise), while Tile freely attaches
#    several. Post-pass: hoist extra waits onto wait-only EventSemaphore
#    instructions inserted immediately before the instruction on the same
#    engine (sequencers execute in order, so the semantics are identical).
_mw_ctr = [0]


def _split_multi_waits(nc: bass.Bass) -> None:
    for f in nc.m.functions:
        for bb in f.blocks:
            out = []
            changed = False
            for inst in bb.instructions:
                si = inst.sync_info
                waits = list(si.on_wait) if si is not None and si.on_wait else []
                if len(waits) > 1:
                    for w in waits[:-1]:
                        _mw_ctr[0] += 1
                        es = mybir.InstEventSemaphore(
                            name=f"I-mwsplit-{_mw_ctr[0]}",
                            engine=inst.engine,
                            ins=[], outs=[],
                            sync_info=mybir.SyncInfo(on_wait=[w], on_update=[]),
                        )
                        out.append(es)
                    inst.sync_info = mybir.SyncInfo(
                        on_wait=[waits[-1]],
                        on_update=list(si.on_update or []),
                    )
                    changed = True
                out.append(inst)
            if changed:
                bb.instructions = out
# -------------------------------------------------------------------------

B = 2048
D = 1024
N = 2 * B                  # 4096 z rows
NCORES = 8
TEMP = 0.05
INV_TEMP = 1.0 / TEMP
BPC = B // NCORES          # 256 image/text rows per core
ZPC = N // NCORES          # 512 z rows per core
NCP = D // 256             # 4 DoubleRow chunk-pairs
BAND = 2048 + 128          # g2i band width per row-tile
ZCOLS = 128 * 3 + BAND     # 2560 local z columns each core touches
NT_L = BPC // 128          # 2 l2i row-tiles per core
NT_G = ZPC // 128          # 4 g2i row-tiles per core
NBLK = ZCOLS // 128        # 20 column blocks for colacc

F8 = mybir.dt.float8e4
BF16 = mybir.dt.bfloat16
F32 = mybir.dt.float32
AF = mybir.ActivationFunctionType
ALU = mybir.AluOpType
DR = mybir.MatmulPerfMode.DoubleRow

# stats_out column layout ([128, 40] f32 per core)
COL_LMAX = 0    # + side*4 + t*2 + h   (8): per-half row max of raw dots
COL_LSUM = 8    # + side*4 + t*2 + h   (8): per-half sum exp(ls*(x-max))
COL_LPOS = 16   # + t                  (2): raw positive dot (unscaled)
COL_GSUM = 18   # + t*3 + {h0,h1,tail} (12): per-part sum exp(sim/temp)
COL_GPOS = 30   # + t                  (4): raw positive cosine sim

# fallback switches for instructions this walrus build rejects ("ISA wrong
# length"): tensor_tensor_reduce and reciprocal_approx_fast both die there.
USE_TTR = False  # tensor_tensor_reduce (else tensor_mul + reduce_sum)
USE_RAF = False  # reciprocal_approx_fast (else rsqrt = exp(-0.5*ln))
USE_STT = True   # scalar_tensor_tensor (else tensor_scalar + tensor_mul)

_cache: dict = {}


def _diag_extract(nc, workp, src, eye, accum):
    """accum[p] = src[p, p] via eye multiply + row reduce."""
    scr = workp.tile([128, 128], F32, tag="scr", bufs=2, name="scr")
    if USE_TTR:
        nc.vector.tensor_tensor_reduce(
            scr, src, eye, 1.0, 0.0, ALU.mult, ALU.add, accum)
    else:
        nc.vector.tensor_mul(scr, src, eye)
        nc.vector.reduce_sum(accum, scr, axis=mybir.AxisListType.X)


def _scale_rowcol(nc, workp, out, ps, rowinv, colinv):
    """out = ps * rowinv (per-partition) * colinv (per-column)."""
    if USE_STT:
        nc.vector.scalar_tensor_tensor(
            out, ps, rowinv, colinv, ALU.mult, ALU.mult)
    else:
        nc.vector.tensor_scalar_mul(out, ps, rowinv)
        nc.vector.tensor_mul(out, out, colinv)


def _build_program(ls: float) -> bass.Bass:
    nc = bass.Bass(trn_type="TRN2")
    # host pre-interleaves to [128, n_chunks*N] (partition-major) so each
    # chunk-pair (or whole tensor) moves in one contiguous DMA descriptor
    img_d = nc.dram_tensor("img", [128, 8 * B], F8, kind="ExternalInput")
    txt_d = nc.dram_tensor("txt", [128, 8 * B], F8, kind="ExternalInput")
    z_d = nc.dram_tensor("z", [128, 8 * ZCOLS], F8, kind="ExternalInput")
    zsq_d = nc.dram_tensor("zsq", [128, 8 * ZCOLS], F8, kind="ExternalInput")
    eye_d = nc.dram_tensor("eye", [128, 128], F32, kind="ExternalInput")
    mask0_d = nc.dram_tensor("mask0", [128, 128], F32, kind="ExternalInput")
    maskt_d = nc.dram_tensor("maskt", [128, 128], F32, kind="ExternalInput")
    stats_d = nc.dram_tensor("stats", [128, 40], F32, kind="ExternalOutput")
    colq_d = nc.dram_tensor("colq", [128, 4 * 17], F32, kind="ExternalOutput")

    with TileContext(nc) as tc:
        with (
            tc.tile_pool(name="consts", bufs=1) as consts,
            tc.tile_pool(name="feat", bufs=NCP) as featp,
            tc.tile_pool(name="escr", bufs=3) as escrp,
            tc.tile_pool(name="work", bufs=4) as workp,
            tc.tile_pool(name="mm", bufs=1, space="PSUM") as mmp,
        ):
            eye = consts.tile([128, 128], F32, tag="eye")
            mask0 = consts.tile([128, 128], F32, tag="mask0")
            maskt = consts.tile([128, 128], F32, tag="maskt")

            ones8 = consts.tile([128, 2, 128], F8, tag="ones8")
            onesb = consts.tile([128, 1], BF16, tag="onesb")
            nc.vector.memset(ones8, 1.0)
            nc.vector.memset(onesb, 1.0)

            stats = consts.tile([128, 40], F32, tag="stats")
            nc.vector.memset(stats, 0.0)

            invn = consts.tile([128, ZCOLS], F32, tag="invn")
            E = consts.tile([128, NT_G, BAND], BF16, tag="E")
            colq_s = consts.tile([128, 4 * 17], F32, tag="colqs")

            # ---- input DMAs ----
            # img/txt: one descriptor per chunk-pair on the sync queue so
            # l2i matmuls can start as pairs land
            img_c = []
            txt_c = []
            for cp in range(NCP):
                it = featp.tile([128, 2, B], F8, tag="img", name="it")
                tt = featp.tile([128, 2, B], F8, tag="tt", name="tt")
                if cp == 0:
                    # split the first pair chunk-wise so PE starts sooner
                    for i in range(2):
                        nc.sync.dma_start(
                            it[:, i, :], img_d[:, i * B:(i + 1) * B])
                        nc.sync.dma_start(
                            tt[:, i, :], txt_d[:, i * B:(i + 1) * B])
                else:
                    nc.sync.dma_start(
                        it, img_d[:, 2 * cp * B:(2 * cp + 2) * B])
                    nc.sync.dma_start(
                        tt, txt_d[:, 2 * cp * B:(2 * cp + 2) * B])
                img_c.append(it)
                txt_c.append(tt)
            # small consts after the first pair (needed only ~mid-l2i)
            nc.sync.dma_start(eye, eye_d[:, :])
            nc.sync.dma_start(mask0, mask0_d[:, :])
            nc.sync.dma_start(maskt, maskt_d[:, :])
            # zsq then z, pair-granular on the same sync queue (a second
            # HWDGE queue or whole-tensor descriptors both starve the
            # small img/txt transfers behind 5MB of z traffic)
            zsq_t = featp.tile([128, 8, ZCOLS], F8, tag="zsq", bufs=1,
                               name="zsq_t")
            z_t = featp.tile([128, 8, ZCOLS], F8, tag="z", bufs=1, name="z_t")
            for cp in range(NCP):
                nc.sync.dma_start(
                    zsq_t[:, 2 * cp:2 * cp + 2, :],
                    zsq_d[:, 2 * cp * ZCOLS:(2 * cp + 2) * ZCOLS])
            for cp in range(NCP):
                nc.sync.dma_start(
                    z_t[:, 2 * cp:2 * cp + 2, :],
                    z_d[:, 2 * cp * ZCOLS:(2 * cp + 2) * ZCOLS])
            zsq_c = [zsq_t[:, 2 * cp:2 * cp + 2, :] for cp in range(NCP)]
            z_c = [z_t[:, 2 * cp:2 * cp + 2, :] for cp in range(NCP)]

            colq = mmp.tile([128, 512], F32, tag="colq", bufs=1, name="colq")

            # ================= Phase A: l2i (two sides) =================
            # Per (side, row-tile, half): [128,1024] psum; per-half max and
            # exp-sum are exported raw and combined into lse on the host.
            for side in range(2):
                lhs_c = img_c if side == 0 else txt_c
                rhs_c = txt_c if side == 0 else img_c
                for t in range(NT_L):
                    for h in range(2):
                        ps = mmp.tile([128, 1024], F32, tag="big", bufs=3,
                                      name="ps")
                        for q in range(2):
                            for cp in range(NCP):
                                nc.tensor.matmul(
                                    ps[:, q * 512:(q + 1) * 512],
                                    lhs_c[cp][:, :, t * 128:(t + 1) * 128],
                                    rhs_c[cp][:, :, h * 1024 + q * 512:
                                              h * 1024 + (q + 1) * 512],
                                    start=(cp == 0), stop=(cp == NCP - 1),
                                    perf_mode=DR,
                                )
                        if side == 0 and h == 0:
                            # raw positive dot: diag of the [128,128] block
                            _diag_extract(
                                nc, workp, ps[:, t * 128:(t + 1) * 128], eye,
                                stats[:, COL_LPOS + t:COL_LPOS + t + 1])
                        col = side * 4 + t * 2 + h
                        mx = stats[:, COL_LMAX + col:COL_LMAX + col + 1]
                        nc.vector.reduce_max(mx, ps, axis=mybir.AxisListType.X)
                        negb = workp.tile([128, 1], F32, tag="negb", name="negb")
                        nc.scalar.mul(negb, mx, -ls)
                        escr = escrp.tile([128, 1024], BF16, tag="escr",
                                          name="escr")
                        nc.scalar.activation(
                            escr, ps, AF.Exp, bias=negb, scale=ls,
                            accum_out=stats[:, COL_LSUM + col:
                                            COL_LSUM + col + 1],
                        )

            # ================= Phase B: z norms =================
            # nsq[j] = sum_d z[d,j]^2 broadcast to all partitions via a
            # DoubleRow ones-matmul over host-shipped zsq; invn = rsqrt.
            nwid = [1024, 1024, 512]
            for i in range(3):
                nb = mmp.tile([128, 1024], F32, tag="big", bufs=3, name="nb")
                for q in range(nwid[i] // 512):
                    off = i * 1024 + q * 512
                    for cp in range(NCP):
                        nc.tensor.matmul(
                            nb[:, q * 512:(q + 1) * 512],
                            ones8,
                            zsq_c[cp][:, :, off:off + 512],
                            start=(cp == 0), stop=(cp == NCP - 1),
                            perf_mode=DR,
                        )
                sl = slice(i * 1024, i * 1024 + nwid[i])
                if USE_RAF:
                    nc.vector.reciprocal_approx_fast(
                        invn[:, sl], nb[:, :nwid[i]])
                    nc.scalar.activation(invn[:, sl], invn[:, sl], AF.Sqrt)
                else:
                    # rsqrt(x) = exp(-0.5*ln(x)) on the accurate ACT tables
                    nc.scalar.activation(invn[:, sl], nb[:, :nwid[i]], AF.Ln)
                    nc.scalar.activation(invn[:, sl], invn[:, sl], AF.Exp,
                                         scale=-0.5)

            # ================= Phase C: g2i (banded symmetric) =================
            for t in range(NT_G):
                # inv-norms of this tile's own rows, in partition layout
                rowinv = workp.tile([128, 1], F32, tag="rinv", bufs=2,
                                    name="rowinv")
                _diag_extract(nc, workp, invn[:, t * 128:(t + 1) * 128],
                              eye, rowinv)
                for h in range(2):
                    ps = mmp.tile([128, 1024], F32, tag="big", bufs=3,
                                  name="ps")
                    for q in range(2):
                        off = t * 128 + h * 1024 + q * 512
                        for cp in range(NCP):
                            nc.tensor.matmul(
                                ps[:, q * 512:(q + 1) * 512],
                                z_c[cp][:, :, t * 128:(t + 1) * 128],
                                z_c[cp][:, :, off:off + 512],
                                start=(cp == 0), stop=(cp == NCP - 1),
                                perf_mode=DR,
                            )
                    # sim = G * rowinv_i * colinv_j, fused STT writing to
                    # SBUF so the psum slot frees after DVE (not after exp)
                    simS = escrp.tile([128, 1024], F32, tag="simS",
                                      name="simS")
                    _scale_rowcol(
                        nc, workp, simS, ps, rowinv,
                        invn[:, t * 128 + h * 1024:t * 128 + (h + 1) * 1024])
                    if h == 0:
                        # self-block: keep strict upper (d in [1,127])
                        nc.vector.tensor_add(
                            simS[:, 0:128], simS[:, 0:128], mask0)
                    col = COL_GSUM + t * 3 + h
                    nc.scalar.activation(
                        E[:, t, h * 1024:(h + 1) * 1024], simS, AF.Exp,
                        scale=INV_TEMP,
                        accum_out=stats[:, col:col + 1],
                    )
                # tail block: cols [2048, 2176) of the band, keep d<=2048
                pt = mmp.tile([128, 512], F32, tag="tail", bufs=1, name="pt")
                off = t * 128 + 2048
                for cp in range(NCP):
                    nc.tensor.matmul(
                        pt[:, 0:128],
                        z_c[cp][:, :, t * 128:(t + 1) * 128],
                        z_c[cp][:, :, off:off + 128],
                        start=(cp == 0), stop=(cp == NCP - 1),
                        perf_mode=DR,
                    )
                tailS = workp.tile([128, 128], F32, tag="tailS", bufs=2,
                                   name="tailS")
                _scale_rowcol(nc, workp, tailS, pt[:, 0:128], rowinv,
                              invn[:, off:off + 128])
                nc.vector.tensor_add(tailS, tailS, maskt)
                # positive pair: diag (d = 2048) of the tail block
                _diag_extract(nc, workp, tailS, eye,
                              stats[:, COL_GPOS + t:COL_GPOS + t + 1])
                col = COL_GSUM + t * 3 + 2
                nc.scalar.activation(
                    E[:, t, 2048:2048 + 128], tailS, AF.Exp,
                    scale=INV_TEMP,
                    accum_out=stats[:, col:col + 1],
                )

            # column sums: weight-stationary matmuls, E block as lhsT, ones
            # column as rhs -> per-(t,block) column in partition layout.
            # Single-shot matmuls (interleaved long-lived psum accumulation
            # groups lose prior contributions on HW); the host sums the 4
            # tile layers. Emitted AFTER all sim tiles: during the midspan
            # the PE is saturated with sim matmuls anyway, and these 68
            # small matmuls are the only PE work available to fill the
            # drain gap while tile 3's STT/exp chain completes.
            for t in range(NT_G):
                for j in range(17):
                    nc.tensor.matmul(
                        colq[:, t * 17 + j:t * 17 + j + 1],
                        E[:, t, j * 128:(j + 1) * 128],
                        onesb,
                    )

            # split the drains so the final chain only waits on tile-3 bits
            nc.scalar.copy(colq_s[:, 0:3 * 17], colq[:, 0:3 * 17])
            nc.sync.dma_start(stats_d[:, 0:COL_GSUM], stats[:, 0:COL_GSUM])
            nc.sync.dma_start(colq_d[:, 0:3 * 17], colq_s[:, 0:3 * 17])
            nc.scalar.copy(colq_s[:, 3 * 17:4 * 17], colq[:, 3 * 17:4 * 17])
            nc.sync.dma_start(stats_d[:, COL_GSUM:], stats[:, COL_GSUM:])
            nc.sync.dma_start(colq_d[:, 3 * 17:4 * 17], colq_s[:, 3 * 17:4 * 17])

    _split_multi_waits(nc)
    return nc


def _get_program(ls: float) -> bass.Bass:
    key = float(ls)
    if key not in _cache:
        _cache[key] = _build_program(key)
    return _cache[key]


def kernel(image_features, gli_features, text_features, logit_scale):
    ls = float(np.asarray(logit_scale))
    nc = _get_program(ls)

    f8 = ml_dtypes.float8_e4m3
    imgT = np.ascontiguousarray(np.asarray(image_features, np.float32).T)
    txtT = np.ascontiguousarray(np.asarray(text_features, np.float32).T)
    z = np.concatenate(
        [np.asarray(gli_features, np.float32),
         np.asarray(image_features, np.float32)], axis=0)
    zT = np.ascontiguousarray(z.T)

    eye = np.eye(128, dtype=np.float32)
    r = np.arange(128)
    # mask0: keep strict upper (s > r); maskt: keep s <= r (incl. diag)
    mask0 = np.where(r[None, :] > r[:, None], 0.0, -1e30).astype(np.float32)
    maskt = np.where(r[None, :] <= r[:, None], 0.0, -1e30).astype(np.float32)

    def interleave(a, ncols):
        # [1024, ncols] -> [128, 8*ncols] partition-major chunk layout
        return np.ascontiguousarray(
            a.reshape(8, 128, ncols).transpose(1, 0, 2).reshape(128, -1))

    in_maps = []
    for k in range(NCORES):
        zr = np.roll(zT, -ZPC * k, axis=1)[:, :ZCOLS].astype(f8)
        zrf = zr.astype(np.float32)
        in_maps.append({
            "img": interleave(np.roll(imgT, -BPC * k, axis=1).astype(f8), B),
            "txt": interleave(np.roll(txtT, -BPC * k, axis=1).astype(f8), B),
            "z": interleave(zr, ZCOLS),
            "zsq": interleave((zrf * zrf).astype(f8), ZCOLS),
            "eye": eye,
            "mask0": mask0,
            "maskt": maskt,
        })

    res = bass_utils.run_bass_kernel_spmd(nc, in_maps, core_ids=list(range(NCORES)))
    globals()["LAST_RESULT"] = res
    stats = np.stack([r_["stats"] for r_ in res.results]).astype(np.float64)
    colq = np.stack([r_["colq"] for r_ in res.results]).astype(np.float64)

    # ---- l2i: combine per-half (max, sumexp) into lse on the host ----
    lse_sum = np.zeros(2)
    for side in range(2):
        for t in range(NT_L):
            cols = [side * 4 + t * 2 + h for h in range(2)]
            m = stats[:, :, [COL_LMAX + c for c in cols]]      # [8,128,2]
            s = stats[:, :, [COL_LSUM + c for c in cols]]
            M = m.max(axis=2)
            comb = (s * np.exp(ls * (m - M[:, :, None]))).sum(axis=2)
            lse_sum[side] += (ls * M + np.log(comb)).sum()
    pos_l2i = stats[:, :, COL_LPOS:COL_LPOS + NT_L].sum()
    l2i = 0.5 * ((lse_sum[0] - ls * pos_l2i) / B
                 + (lse_sum[1] - ls * pos_l2i) / B)

    # ---- g2i: assemble row sums from row partials + column sums ----
    # per-core row partials rowacc[k, local_row]
    rowacc = np.zeros((NCORES, ZPC))
    pos = np.zeros((NCORES, ZPC))
    for t in range(NT_G):
        sl = slice(t * 128, (t + 1) * 128)
        rowacc[:, sl] = stats[:, :, COL_GSUM + t * 3:COL_GSUM + t * 3 + 3] \
            .sum(axis=2)
        pos[:, sl] = stats[:, :, COL_GPOS + t]
    # column sums: colq[k, i, t*17+j] is local column 128*(t+j) + i
    colsum = np.zeros(N)
    for k in range(NCORES):
        local = np.zeros(ZCOLS)
        for t in range(NT_G):
            lc = colq[k][:, t * 17:(t + 1) * 17]   # [128, 17]
            local[128 * t:128 * t + 2176] += lc.T.reshape(-1)
        gidx = (ZPC * k + np.arange(ZCOLS)) % N
        np.add.at(colsum, gidx, local)
    rows = rowacc.reshape(-1)
    posf = pos.reshape(-1)
    total = rows + colsum - np.exp(INV_TEMP * posf)
    lse = np.log(total)
    g2i = (lse - INV_TEMP * posf).sum() / N

    total_loss = l2i + g2i
    return (np.float32(total_loss), np.float32(l2i), np.float32(g2i))


# revision 44
# speedup vs baseline: 1.0671x; 1.0366x over previous
"""Distributed CLIP-style loss (l2i symmetric CE + g2i NT-Xent) on 8 TRN2 cores.

v2: fp8 DoubleRow matmuls + circulant-banded symmetric g2i.

Each core k owns 256 l2i rows and 512 z rows. Inputs are column-ROTATED
transposed matrices (rotation = the core's global row offset), so one SPMD
program serves all 8 cores and the similarity structure becomes circulant:
the "upper triangle" of the symmetric (4096,4096) g2i sim matrix is, for
row-tile t, the contiguous local column band [128t, 128t+2176). Each pair
{i,j} is computed exactly once (d=(j-i)%4096 in [1,2048]); row-sums get the
missing lower-triangle terms from column sums of the exp matrix, assembled
on the host (d=2048 positive pairs land twice and are subtracted there).

g2i runs on RAW fp8 z (sim = G * rowinv_i * colinv_j applied after the
matmul with one fused scalar_tensor_tensor per psum half), so the sim
matmuls do not wait for the normalization pipeline. Norms come from a
host-shipped fp8 z^2 tensor via a DoubleRow ones-matmul + fast reciprocal.
Column sums of exp use weight-stationary matmuls (E-block as lhsT, ones
column as rhs) accumulating into a single [128,20] psum column so they
come out in partition layout with no cross-bank drains.
"""

import numpy as np
import ml_dtypes

import concourse.bass as bass
import concourse.mybir as mybir
from concourse.tile import TileContext
from concourse import bass_utils


# --- compat patches for the walrus build in this container ---------------
# 1) EVENT_SEMAPHORE_RANGE_CLEAR (InstISA op 176) is rejected ("ISA wrong
#    length"); emit one EventSemaphore sem-wr-imm 0 per semaphore instead.
SEM_CLEAR_BATCH = 1  # walrus rejects >1 sem update per EventSemaphore


def _sem_clear_compat(self, sem):
    nums = list(sem) if isinstance(sem, range) else [
        sem.num if hasattr(sem, "num") else int(sem)
    ]
    last = None
    for i in range(0, len(nums), SEM_CLEAR_BATCH):
        last = self.add_instruction(
            mybir.InstEventSemaphore(
                name=self.bass.get_next_instruction_name(),
                ins=[], outs=[],
                sync_info=mybir.SyncInfo(
                    on_wait=[],
                    on_update=[mybir.SyncUpdate(
                        sync_type="semaphore", id=n,
                        update_mode="sem-wr-imm", update_value=0)
                        for n in nums[i:i + SEM_CLEAR_BATCH]],
                ),
            )
        )
    return last


bass.BassGpSimd.sem_clear = _sem_clear_compat


# 2) Every instruction in this walrus build has a single sync-wait slot
#    ("Too many sync wait commands" otherwise), while Tile freely attaches
#    several. Post-pass: hoist extra waits onto wait-only EventSemaphore
#    instructions inserted immediately before the instruction on the same
#    engine (sequencers execute in order, so the semantics are identical).
_mw_ctr = [0]


def _split_multi_waits(nc: bass.Bass) -> None:
    for f in nc.m.functions:
        for bb in f.blocks:
            out = []
            changed = False
            for inst in bb.instructions:
                si = inst.sync_info
                waits = list(si.on_wait) if si is not None and si.on_wait else []
                if len(waits) > 1:
                    for w in waits[:-1]:
                        _mw_ctr[0] += 1
                        es = mybir.InstEventSemaphore(
                            name=f"I-mwsplit-{_mw_ctr[0]}",
                            engine=inst.engine,
                            ins=[], outs=[],
                            sync_info=mybir.SyncInfo(on_wait=[w], on_update=[]),
                        )
                        out.append(es)
                    inst.sync_info = mybir.SyncInfo(
                        on_wait=[waits[-1]],
                        on_update=list(si.on_update or []),
                    )
                    changed = True
                out.append(inst)
            if changed:
                bb.instructions = out
# -------------------------------------------------------------------------

B = 2048
D = 1024
N = 2 * B                  # 4096 z rows
NCORES = 8
TEMP = 0.05
INV_TEMP = 1.0 / TEMP
BPC = B // NCORES          # 256 image/text rows per core
ZPC = N // NCORES          # 512 z rows per core
NCP = D // 256             # 4 DoubleRow chunk-pairs
BAND = 2048 + 128          # g2i band width per row-tile
ZCOLS = 128 * 3 + BAND     # 2560 local z columns each core touches
NT_L = BPC // 128          # 2 l2i row-tiles per core
NT_G = ZPC // 128          # 4 g2i row-tiles per core
NBLK = ZCOLS // 128        # 20 column blocks for colacc

F8 = mybir.dt.float8e4
BF16 = mybir.dt.bfloat16
F32 = mybir.dt.float32
AF = mybir.ActivationFunctionType
ALU = mybir.AluOpType
DR = mybir.MatmulPerfMode.DoubleRow

# stats_out column layout ([128, 40] f32 per core)
COL_LMAX = 0    # + side*4 + t*2 + h   (8): per-half row max of raw dots
COL_LSUM = 8    # + side*4 + t*2 + h   (8): per-half sum exp(ls*(x-max))
COL_LPOS = 16   # + t                  (2): raw positive dot (unscaled)
COL_GSUM = 18   # + t*3 + {h0,h1,tail} (12): per-part sum exp(sim/temp)
COL_GPOS = 30   # + t                  (4): raw positive cosine sim

# fallback switches for instructions this walrus build rejects ("ISA wrong
# length"): tensor_tensor_reduce and reciprocal_approx_fast both die there.
USE_TTR = False  # tensor_tensor_reduce (else tensor_mul + reduce_sum)
USE_RAF = False  # reciprocal_approx_fast (else rsqrt = exp(-0.5*ln))
USE_STT = True   # scalar_tensor_tensor (else tensor_scalar + tensor_mul)

_cache: dict = {}


def _diag_extract(nc, workp, src, eye, accum):
    """accum[p] = src[p, p] via eye multiply + row reduce."""
    scr = workp.tile([128, 128], F32, tag="scr", bufs=2, name="scr")
    if USE_TTR:
        nc.vector.tensor_tensor_reduce(
            scr, src, eye, 1.0, 0.0, ALU.mult, ALU.add, accum)
    else:
        nc.vector.tensor_mul(scr, src, eye)
        nc.vector.reduce_sum(accum, scr, axis=mybir.AxisListType.X)


def _scale_rowcol(nc, workp, out, ps, rowinv, colinv):
    """out = ps * rowinv (per-partition) * colinv (per-column)."""
    if USE_STT:
        nc.vector.scalar_tensor_tensor(
            out, ps, rowinv, colinv, ALU.mult, ALU.mult)
    else:
        nc.vector.tensor_scalar_mul(out, ps, rowinv)
        nc.vector.tensor_mul(out, out, colinv)


def _build_program(ls: float) -> bass.Bass:
    nc = bass.Bass(trn_type="TRN2")
    # host pre-interleaves to [128, n_chunks*N] (partition-major) so each
    # chunk-pair (or whole tensor) moves in one contiguous DMA descriptor
    img_d = nc.dram_tensor("img", [128, 8 * B], F8, kind="ExternalInput")
    txt_d = nc.dram_tensor("txt", [128, 8 * B], F8, kind="ExternalInput")
    z_d = nc.dram_tensor("z", [128, 8 * ZCOLS], F8, kind="ExternalInput")
    zsq_d = nc.dram_tensor("zsq", [128, 8 * ZCOLS], F8, kind="ExternalInput")
    eye_d = nc.dram_tensor("eye", [128, 128], F32, kind="ExternalInput")
    mask0_d = nc.dram_tensor("mask0", [128, 128], F32, kind="ExternalInput")
    maskt_d = nc.dram_tensor("maskt", [128, 128], F32, kind="ExternalInput")
    stats_d = nc.dram_tensor("stats", [128, 40], F32, kind="ExternalOutput")
    colq_d = nc.dram_tensor("colq", [128, 4 * 17], F32, kind="ExternalOutput")

    with TileContext(nc) as tc:
        with (
            tc.tile_pool(name="consts", bufs=1) as consts,
            tc.tile_pool(name="feat", bufs=NCP) as featp,
            tc.tile_pool(name="escr", bufs=3) as escrp,
            tc.tile_pool(name="work", bufs=4) as workp,
            tc.tile_pool(name="mm", bufs=1, space="PSUM") as mmp,
        ):
            eye = consts.tile([128, 128], F32, tag="eye")
            mask0 = consts.tile([128, 128], F32, tag="mask0")
            maskt = consts.tile([128, 128], F32, tag="maskt")

            ones8 = consts.tile([128, 2, 128], F8, tag="ones8")
            onesb = consts.tile([128, 1], BF16, tag="onesb")
            nc.vector.memset(ones8, 1.0)
            nc.vector.memset(onesb, 1.0)

            stats = consts.tile([128, 40], F32, tag="stats")
            nc.vector.memset(stats, 0.0)

            invn = consts.tile([128, ZCOLS], F32, tag="invn")
            E = consts.tile([128, NT_G, BAND], BF16, tag="E")
            colq_s = consts.tile([128, 4 * 17], F32, tag="colqs")

            # ---- input DMAs ----
            # zsq FIRST: the norm matmul is the only PE work independent of
            # img/txt, so it fills (and HAM-warms) the DMA-gated ramp that
            # otherwise idles at K=4 waiting for l2i pairs.
            zsq_t = featp.tile([128, 8, ZCOLS], F8, tag="zsq", bufs=1,
                               name="zsq_t")
            z_t = featp.tile([128, 8, ZCOLS], F8, tag="z", bufs=1, name="z_t")
            for cp in range(NCP):
                nc.sync.dma_start(
                    zsq_t[:, 2 * cp:2 * cp + 2, :],
                    zsq_d[:, 2 * cp * ZCOLS:(2 * cp + 2) * ZCOLS])
            # small consts next (needed ~mid-l2i)
            nc.sync.dma_start(eye, eye_d[:, :])
            nc.sync.dma_start(mask0, mask0_d[:, :])
            nc.sync.dma_start(maskt, maskt_d[:, :])
            # img/txt: one descriptor per chunk-pair so l2i follows norm
            img_c = []
            txt_c = []
            for cp in range(NCP):
                it = featp.tile([128, 2, B], F8, tag="img", name="it")
                tt = featp.tile([128, 2, B], F8, tag="tt", name="tt")
                nc.sync.dma_start(
                    it, img_d[:, 2 * cp * B:(2 * cp + 2) * B])
                nc.sync.dma_start(
                    tt, txt_d[:, 2 * cp * B:(2 * cp + 2) * B])
                img_c.append(it)
                txt_c.append(tt)
            # z last: sim matmuls only reach its final pair ~mid-kernel
            for cp in range(NCP):
                nc.sync.dma_start(
                    z_t[:, 2 * cp:2 * cp + 2, :],
                    z_d[:, 2 * cp * ZCOLS:(2 * cp + 2) * ZCOLS])
            zsq_c = [zsq_t[:, 2 * cp:2 * cp + 2, :] for cp in range(NCP)]
            z_c = [z_t[:, 2 * cp:2 * cp + 2, :] for cp in range(NCP)]

            colq = mmp.tile([128, 512], F32, tag="colq", bufs=1, name="colq")

            # ================= Phase B: z norms (emitted FIRST) =========
            # nsq[j] = sum_d z[d,j]^2 broadcast to all partitions via a
            # DoubleRow ones-matmul over host-shipped zsq; invn = rsqrt.
            # Runs during the ramp while img/txt stream in.
            nwid = [1024, 1024, 512]
            for i in range(3):
                nb = mmp.tile([128, 1024], F32, tag="big", bufs=3, name="nb")
                for q in range(nwid[i] // 512):
                    off = i * 1024 + q * 512
                    for cp in range(NCP):
                        nc.tensor.matmul(
                            nb[:, q * 512:(q + 1) * 512],
                            ones8,
                            zsq_c[cp][:, :, off:off + 512],
                            start=(cp == 0), stop=(cp == NCP - 1),
                            perf_mode=DR,
                        )
                sl = slice(i * 1024, i * 1024 + nwid[i])
                if USE_RAF:
                    nc.vector.reciprocal_approx_fast(
                        invn[:, sl], nb[:, :nwid[i]])
                    nc.scalar.activation(invn[:, sl], invn[:, sl], AF.Sqrt)
                else:
                    # rsqrt(x) = exp(-0.5*ln(x)) on the accurate ACT tables
                    nc.scalar.activation(invn[:, sl], nb[:, :nwid[i]], AF.Ln)
                    nc.scalar.activation(invn[:, sl], invn[:, sl], AF.Exp,
                                         scale=-0.5)

            # ================= Phase A: l2i (two sides) =================
            # Per (side, row-tile, half): [128,1024] psum; per-half max and
            # exp-sum are exported raw and combined into lse on the host.
            for side in range(2):
                lhs_c = img_c if side == 0 else txt_c
                rhs_c = txt_c if side == 0 else img_c
                for t in range(NT_L):
                    for h in range(2):
                        ps = mmp.tile([128, 1024], F32, tag="big", bufs=3,
                                      name="ps")
                        for q in range(2):
                            for cp in range(NCP):
                                nc.tensor.matmul(
                                    ps[:, q * 512:(q + 1) * 512],
                                    lhs_c[cp][:, :, t * 128:(t + 1) * 128],
                                    rhs_c[cp][:, :, h * 1024 + q * 512:
                                              h * 1024 + (q + 1) * 512],
                                    start=(cp == 0), stop=(cp == NCP - 1),
                                    perf_mode=DR,
                                )
                        if side == 0 and h == 0:
                            # raw positive dot: diag of the [128,128] block
                            _diag_extract(
                                nc, workp, ps[:, t * 128:(t + 1) * 128], eye,
                                stats[:, COL_LPOS + t:COL_LPOS + t + 1])
                        col = side * 4 + t * 2 + h
                        mx = stats[:, COL_LMAX + col:COL_LMAX + col + 1]
                        nc.vector.reduce_max(mx, ps, axis=mybir.AxisListType.X)
                        negb = workp.tile([128, 1], F32, tag="negb", name="negb")
                        nc.scalar.mul(negb, mx, -ls)
                        escr = escrp.tile([128, 1024], BF16, tag="escr",
                                          name="escr")
                        nc.scalar.activation(
                            escr, ps, AF.Exp, bias=negb, scale=ls,
                            accum_out=stats[:, COL_LSUM + col:
                                            COL_LSUM + col + 1],
                        )

            # ================= Phase C: g2i (banded symmetric) =================
            for t in range(NT_G):
                # inv-norms of this tile's own rows, in partition layout
                rowinv = workp.tile([128, 1], F32, tag="rinv", bufs=2,
                                    name="rowinv")
                _diag_extract(nc, workp, invn[:, t * 128:(t + 1) * 128],
                              eye, rowinv)
                for h in range(2):
                    ps = mmp.tile([128, 1024], F32, tag="big", bufs=3,
                                  name="ps")
                    for q in range(2):
                        off = t * 128 + h * 1024 + q * 512
                        for cp in range(NCP):
                            nc.tensor.matmul(
                                ps[:, q * 512:(q + 1) * 512],
                                z_c[cp][:, :, t * 128:(t + 1) * 128],
                                z_c[cp][:, :, off:off + 512],
                                start=(cp == 0), stop=(cp == NCP - 1),
                                perf_mode=DR,
                            )
                    # sim = G * rowinv_i * colinv_j, fused STT writing to
                    # SBUF so the psum slot frees after DVE (not after exp)
                    simS = escrp.tile([128, 1024], F32, tag="simS",
                                      name="simS")
                    _scale_rowcol(
                        nc, workp, simS, ps, rowinv,
                        invn[:, t * 128 + h * 1024:t * 128 + (h + 1) * 1024])
                    if h == 0:
                        # self-block: keep strict upper (d in [1,127])
                        nc.vector.tensor_add(
                            simS[:, 0:128], simS[:, 0:128], mask0)
                    col = COL_GSUM + t * 3 + h
                    nc.scalar.activation(
                        E[:, t, h * 1024:(h + 1) * 1024], simS, AF.Exp,
                        scale=INV_TEMP,
                        accum_out=stats[:, col:col + 1],
                    )
                # tail block: cols [2048, 2176) of the band, keep d<=2048
                pt = mmp.tile([128, 512], F32, tag="tail", bufs=1, name="pt")
                off = t * 128 + 2048
                for cp in range(NCP):
                    nc.tensor.matmul(
                        pt[:, 0:128],
                        z_c[cp][:, :, t * 128:(t + 1) * 128],
                        z_c[cp][:, :, off:off + 128],
                        start=(cp == 0), stop=(cp == NCP - 1),
                        perf_mode=DR,
                    )
                tailS = workp.tile([128, 128], F32, tag="tailS", bufs=2,
                                   name="tailS")
                _scale_rowcol(nc, workp, tailS, pt[:, 0:128], rowinv,
                              invn[:, off:off + 128])
                nc.vector.tensor_add(tailS, tailS, maskt)
                # positive pair: diag (d = 2048) of the tail block
                _diag_extract(nc, workp, tailS, eye,
                              stats[:, COL_GPOS + t:COL_GPOS + t + 1])
                col = COL_GSUM + t * 3 + 2
                nc.scalar.activation(
                    E[:, t, 2048:2048 + 128], tailS, AF.Exp,
                    scale=INV_TEMP,
                    accum_out=stats[:, col:col + 1],
                )

                # column sums: weight-stationary matmuls, E block as lhsT,
                # ones column as rhs -> per-(t,block) column in partition
                # layout. Single-shot matmuls (interleaved long-lived psum
                # accumulation groups lose prior contributions on HW); the
                # host sums the 4 tile layers.
                for j in range(17):
                    nc.tensor.matmul(
                        colq[:, t * 17 + j:t * 17 + j + 1],
                        E[:, t, j * 128:(j + 1) * 128],
                        onesb,
                    )

            # split the drains so the final chain only waits on tile-3 bits
            nc.scalar.copy(colq_s[:, 0:3 * 17], colq[:, 0:3 * 17])
            nc.sync.dma_start(stats_d[:, 0:COL_GSUM], stats[:, 0:COL_GSUM])
            nc.sync.dma_start(colq_d[:, 0:3 * 17], colq_s[:, 0:3 * 17])
            nc.scalar.copy(colq_s[:, 3 * 17:4 * 17], colq[:, 3 * 17:4 * 17])
            nc.sync.dma_start(stats_d[:, COL_GSUM:], stats[:, COL_GSUM:])
            nc.sync.dma_start(colq_d[:, 3 * 17:4 * 17], colq_s[:, 3 * 17:4 * 17])

    _split_multi_waits(nc)
    return nc


def _get_program(ls: float) -> bass.Bass:
    key = float(ls)
    if key not in _cache:
        _cache[key] = _build_program(key)
    return _cache[key]


def kernel(image_features, gli_features, text_features, logit_scale):
    ls = float(np.asarray(logit_scale))
    nc = _get_program(ls)

    f8 = ml_dtypes.float8_e4m3
    imgT = np.ascontiguousarray(np.asarray(image_features, np.float32).T)
    txtT = np.ascontiguousarray(np.asarray(text_features, np.float32).T)
    z = np.concatenate(
        [np.asarray(gli_features, np.float32),
         np.asarray(image_features, np.float32)], axis=0)
    zT = np.ascontiguousarray(z.T)

    eye = np.eye(128, dtype=np.float32)
    r = np.arange(128)
    # mask0: keep strict upper (s > r); maskt: keep s <= r (incl. diag)
    mask0 = np.where(r[None, :] > r[:, None], 0.0, -1e30).astype(np.float32)
    maskt = np.where(r[None, :] <= r[:, None], 0.0, -1e30).astype(np.float32)

    def interleave(a, ncols):
        # [1024, ncols] -> [128, 8*ncols] partition-major chunk layout
        return np.ascontiguousarray(
            a.reshape(8, 128, ncols).transpose(1, 0, 2).reshape(128, -1))

    in_maps = []
    for k in range(NCORES):
        zr = np.roll(zT, -ZPC * k, axis=1)[:, :ZCOLS].astype(f8)
        zrf = zr.astype(np.float32)
        in_maps.append({
            "img": interleave(np.roll(imgT, -BPC * k, axis=1).astype(f8), B),
            "txt": interleave(np.roll(txtT, -BPC * k, axis=1).astype(f8), B),
            "z": interleave(zr, ZCOLS),
            "zsq": interleave((zrf * zrf).astype(f8), ZCOLS),
            "eye": eye,
            "mask0": mask0,
            "maskt": maskt,
        })

    res = bass_utils.run_bass_kernel_spmd(nc, in_maps, core_ids=list(range(NCORES)))
    globals()["LAST_RESULT"] = res
    stats = np.stack([r_["stats"] for r_ in res.results]).astype(np.float64)
    colq = np.stack([r_["colq"] for r_ in res.results]).astype(np.float64)

    # ---- l2i: combine per-half (max, sumexp) into lse on the host ----
    lse_sum = np.zeros(2)
    for side in range(2):
        for t in range(NT_L):
            cols = [side * 4 + t * 2 + h for h in range(2)]
            m = stats[:, :, [COL_LMAX + c for c in cols]]      # [8,128,2]
            s = stats[:, :, [COL_LSUM + c for c in cols]]
            M = m.max(axis=2)
            comb = (s * np.exp(ls * (m - M[:, :, None]))).sum(axis=2)
            lse_sum[side] += (ls * M + np.log(comb)).sum()
    pos_l2i = stats[:, :, COL_LPOS:COL_LPOS + NT_L].sum()
    l2i = 0.5 * ((lse_sum[0] - ls * pos_l2i) / B
                 + (lse_sum[1] - ls * pos_l2i) / B)

    # ---- g2i: assemble row sums from row partials + column sums ----
    # per-core row partials rowacc[k, local_row]
    rowacc = np.zeros((NCORES, ZPC))
    pos = np.zeros((NCORES, ZPC))
    for t in range(NT_G):
        sl = slice(t * 128, (t + 1) * 128)
        rowacc[:, sl] = stats[:, :, COL_GSUM + t * 3:COL_GSUM + t * 3 + 3] \
            .sum(axis=2)
        pos[:, sl] = stats[:, :, COL_GPOS + t]
    # column sums: colq[k, i, t*17+j] is local column 128*(t+j) + i
    colsum = np.zeros(N)
    for k in range(NCORES):
        local = np.zeros(ZCOLS)
        for t in range(NT_G):
            lc = colq[k][:, t * 17:(t + 1) * 17]   # [128, 17]
            local[128 * t:128 * t + 2176] += lc.T.reshape(-1)
        gidx = (ZPC * k + np.arange(ZCOLS)) % N
        np.add.at(colsum, gidx, local)
    rows = rowacc.reshape(-1)
    posf = pos.reshape(-1)
    total = rows + colsum - np.exp(INV_TEMP * posf)
    lse = np.log(total)
    g2i = (lse - INV_TEMP * posf).sum() / N

    total_loss = l2i + g2i
    return (np.float32(total_loss), np.float32(l2i), np.float32(g2i))
